# revision 1
# baseline (speedup 1.0000x reference)
"""MoE (E=4 experts, top-2 routing) forward pass on 8 Trainium2 NeuronCores.

Strategy: data-parallel over tokens. Full input x is [8, 2048, 1024]; core i
processes batch row i (2048 tokens). Expert weights are replicated to every
core. All experts are computed densely per token (E=4, top-2 -> 2x extra
matmul work, but no data-dependent routing), then combined with the top-2
softmax weights.

Per-core pipeline (T=2048 tokens, D=1024, E=4):
  prologue: PE-transpose x into x^T (bf16 for matmul lhsT + fp32 for gating),
            fp32 gate matmuls -> top-2 softmax weights (fp32: routing needs
            fp32 precision; min top2/top3 score gap on real data is ~2e-5).
  per (expert, token-tile):
    z    = x @ W1 + b1        PE, bf16 operands, fp32 PSUM (b1 via K=1 matmul)
    LN1 stats                 DVE bn_stats/bn_aggr reading PSUM
    n1   = (z - m)*rstd       ACT (per-partition scale/bias), PSUM -> SBUF
    n1  *= g1                 DVE tensor_tensor (in-place)
    n1  += be1                GPSIMD tensor_tensor (in-place)
    u    = relu(n1)           ACT, bf16 out
    u^T                       PE transpose (8x 128x128 bf16 blocks)
    z2   = u @ W2 + b2        PE bf16
    LN2 stats                 DVE
    n2   = (z2 - m2)*rstd2*w_e  ACT (w_e folded into the scale)
    n2  *= g2                 DVE (in-place)
    acc += n2                 GPSIMD (e=0 does acc = n2 + x residual)
  finalize per token-tile:
    C    = w @ be2            PE (K=4 matmul over experts)
    out  = C + acc            DVE, then DMA out
"""

import threading

import numpy as np

import concourse.bass as bass
import concourse.mybir as mybir
import concourse.tile as tile
from concourse import bacc
from concourse.bass import ds, ts
from concourse.masks import make_identity

F32 = mybir.dt.float32
BF16 = mybir.dt.bfloat16
AF = mybir.ActivationFunctionType
ALU = mybir.AluOpType
AX = mybir.AxisListType

P = 128
D = 1024
E = 4
KC = D // P  # contraction chunks per matmul
NCH = D // 512  # psum column chunks
LN_EPS = 1e-5
N_CORES = 8


def _row1(ap):
    """Lift an AP to have a leading length-1 (partition) dim."""
    return bass.AP(tensor=ap.tensor, offset=ap.offset, ap=[[0, 1]] + list(ap.ap))


def _bcast_rows(ap_row, p=P):
    """Broadcast a [1, N]-ish DRAM AP across p partitions (step-0 partition dim)."""
    inner = [list(d) for d in ap_row.ap if d[1] != 1]
    return bass.AP(tensor=ap_row.tensor, offset=ap_row.offset, ap=[[0, p]] + inner)


def build_moe_nc(T=2048, num_devices=N_CORES):
    TT = T // P
    nc = bacc.Bacc(
        "TRN2", target_bir_lowering=False, debug=False, num_devices=num_devices
    )

    x_d = nc.dram_tensor("x", [T, D], F32, kind="ExternalInput")
    gw_d = nc.dram_tensor("gate_W", [D, E], F32, kind="ExternalInput")
    gb_d = nc.dram_tensor("gate_b", [E], F32, kind="ExternalInput")
    w1_d = nc.dram_tensor("W1", [E, D, D], F32, kind="ExternalInput")
    b1_d = nc.dram_tensor("b1", [E, D], F32, kind="ExternalInput")
    g1_d = nc.dram_tensor("g1", [E, D], F32, kind="ExternalInput")
    be1_d = nc.dram_tensor("be1", [E, D], F32, kind="ExternalInput")
    w2_d = nc.dram_tensor("W2", [E, D, D], F32, kind="ExternalInput")
    b2_d = nc.dram_tensor("b2", [E, D], F32, kind="ExternalInput")
    g2_d = nc.dram_tensor("g2", [E, D], F32, kind="ExternalInput")
    be2_d = nc.dram_tensor("be2", [E, D], F32, kind="ExternalInput")
    out_d = nc.dram_tensor("out", [T, D], F32, kind="ExternalOutput")

    with tile.TileContext(nc) as tc:
        with (
            tc.tile_pool(name="const", bufs=1) as const,
            tc.tile_pool(name="w1p", bufs=12) as w1p,
            tc.tile_pool(name="w2p", bufs=12) as w2p,
            tc.tile_pool(name="repp", bufs=2) as repp,
            tc.tile_pool(name="bvep", bufs=2) as bvep,
            tc.tile_pool(name="accp", bufs=TT) as accp,
            tc.tile_pool(name="workp", bufs=2) as workp,
            tc.tile_pool(name="xinp", bufs=2) as xinp,
            tc.tile_pool(name="statp", bufs=3) as statp,
            tc.tile_pool(name="gstp", bufs=1) as gstp,
        ):
            # ---- constants ----
            id_f32 = const.tile([P, P], F32)
            make_identity(nc, id_f32)
            id_bf16 = const.tile([P, P], BF16)
            make_identity(nc, id_bf16)
            ones_bf = const.tile([1, P], BF16)
            nc.vector.memset(ones_bf, 1.0)
            ones_f32 = const.tile([1, P], F32)
            nc.vector.memset(ones_f32, 1.0)
            eps_sb = const.tile([P, 1], F32)
            nc.vector.memset(eps_sb, LN_EPS)

            gw_sb = const.tile([P, KC, E], F32)
            nc.sync.dma_start(out=gw_sb, in_=gw_d.rearrange("(c p) e -> p c e", p=P))
            gb_sb = const.tile([1, E], F32)
            nc.sync.dma_start(out=gb_sb, in_=_row1(gb_d[:]))

            be2_sb = const.tile([E, D], BF16)
            nc.gpsimd.dma_start(out=be2_sb, in_=be2_d[:, :])  # casting dma

            xt_sb = const.tile([P, KC, T], BF16)  # x^T, matmul lhsT layout
            scores_sb = const.tile([P, TT, E], F32)
            w_sb = const.tile([P, TT, E], F32)
            wT_sb = const.tile([E, TT, P], BF16)

            w1tiles = {}
            w2tiles = {}
            bves = {}

            def load_w_chunk(e, c):
                t1w = w1p.tile([P, D], BF16, tag="w1", name=f"w1_{e}_{c}")
                nc.gpsimd.dma_start(out=t1w, in_=w1_d[e, ts(c, P), :])
                w1tiles[(e, c)] = t1w
                t2w = w2p.tile([P, D], BF16, tag="w2", name=f"w2_{e}_{c}")
                nc.gpsimd.dma_start(out=t2w, in_=w2_d[e, ts(c, P), :])
                w2tiles[(e, c)] = t2w

            for _c in range(KC):
                load_w_chunk(0, _c)

            # ---- prologue: transpose x, gate scores ----
            pre_ctx = tc.tile_pool(name="prep", bufs=2, space="PSUM")
            prep = pre_ctx.__enter__()
            for tt in range(TT):
                xin = xinp.tile([P, D], F32, tag="xin")
                nc.sync.dma_start(out=xin, in_=x_d[ts(tt, P), :])
                tp = prep.tile([P, D], F32, tag="tp")
                for c in range(KC):
                    nc.tensor.transpose(tp[:, ts(c, P)], xin[:, ts(c, P)], id_f32)
                xtg = workp.tile([P, D], F32, tag="n1")
                nc.scalar.copy(out=xtg, in_=tp)
                nc.vector.tensor_copy(
                    out=xt_sb[:, :, ts(tt, P)],
                    in_=tp.rearrange("p (c q) -> p c q", c=KC),
                )
                gps = prep.tile([P, E], F32, tag="gate")
                for c in range(KC):
                    nc.tensor.matmul(
                        gps,
                        xtg[:, ts(c, P)],
                        gw_sb[:, c, :],
                        start=(c == 0),
                        stop=False,
                    )
                nc.tensor.matmul(gps, ones_f32, gb_sb, start=False, stop=True)
                nc.scalar.copy(out=scores_sb[:, tt, :], in_=gps)

            # ---- top-2 softmax over the E=4 scores ----
            s3 = scores_sb  # [P, TT, E]
            m1 = gstp.tile([P, TT], F32, tag="m1")
            nc.vector.tensor_reduce(out=m1, in_=s3, axis=AX.X, op=ALU.max)
            m1b = m1.broadcast_to((P, TT, E))
            eqt = gstp.tile([P, TT, E], F32, tag="eqt")
            nc.vector.tensor_tensor(out=eqt, in0=s3, in1=m1b, op=ALU.is_equal)
            smt = gstp.tile([P, TT, E], F32, tag="smt")
            nc.vector.scalar_tensor_tensor(
                out=smt, in0=eqt, scalar=-1e30, in1=s3, op0=ALU.mult, op1=ALU.add
            )
            m2 = gstp.tile([P, TT], F32, tag="m2")
            nc.vector.tensor_reduce(out=m2, in_=smt, axis=AX.X, op=ALU.max)
            m2b = m2.broadcast_to((P, TT, E))
            ind = gstp.tile([P, TT, E], F32, tag="ind")
            nc.vector.tensor_tensor(out=ind, in0=s3, in1=m2b, op=ALU.is_ge)
            dd = gstp.tile([P, TT, E], F32, tag="dd")
            nc.vector.tensor_tensor(out=dd, in0=s3, in1=m1b, op=ALU.subtract)
            ex = gstp.tile([P, TT, E], F32, tag="ex")
            nc.scalar.activation(out=ex, in_=dd, func=AF.Exp)
            en = gstp.tile([P, TT, E], F32, tag="en")
            nc.vector.tensor_tensor(out=en, in0=ex, in1=ind, op=ALU.mult)
            zs = gstp.tile([P, TT], F32, tag="zs")
            nc.vector.tensor_reduce(out=zs, in_=en, axis=AX.X, op=ALU.add)
            rz = gstp.tile([P, TT], F32, tag="rz")
            nc.vector.reciprocal(out=rz, in_=zs)
            rzb = rz.broadcast_to((P, TT, E))
            nc.vector.tensor_tensor(out=w_sb, in0=en, in1=rzb, op=ALU.mult)
            for tt in range(TT):
                wtp = prep.tile([E, P], F32, tag="gate")
                nc.tensor.transpose(wtp, w_sb[:, tt, :], id_f32)
                nc.scalar.copy(out=wT_sb[:, tt, :], in_=wtp)

            pre_ctx.__exit__(None, None, None)
            zp_ctx = tc.tile_pool(name="zp", bufs=2, space="PSUM")
            zp = zp_ctx.__enter__()
            z2p_ctx = tc.tile_pool(name="z2p", bufs=1, space="PSUM")
            z2p = z2p_ctx.__enter__()
            utp_ctx = tc.tile_pool(name="utp", bufs=2, space="PSUM")
            utp = utp_ctx.__enter__()

            # ---- dense expert loop ----
            acc = {}

            def load_bve(e):
                bve = bvep.tile([1, 2, D], BF16, tag="bve", name=f"bve_{e}")
                nc.gpsimd.dma_start(out=bve[:, 0, :], in_=_row1(b1_d[e, :]))
                nc.gpsimd.dma_start(out=bve[:, 1, :], in_=_row1(b2_d[e, :]))
                bves[e] = bve

            reps = {}

            def load_reps(e):
                g1r = repp.tile([P, D], BF16, tag="g1r", name=f"g1r_{e}")
                nc.gpsimd.dma_start(out=g1r, in_=_bcast_rows(g1_d[e : e + 1, :]))
                be1r = repp.tile([P, D], BF16, tag="be1r", name=f"be1r_{e}")
                nc.gpsimd.dma_start(out=be1r, in_=_bcast_rows(be1_d[e : e + 1, :]))
                g2r = repp.tile([P, D], BF16, tag="g2r", name=f"g2r_{e}")
                nc.gpsimd.dma_start(out=g2r, in_=_bcast_rows(g2_d[e : e + 1, :]))
                reps[e] = (g1r, be1r, g2r)

            PREFETCH = 4  # chunks of expert e+1 issued inside expert e's loop
            for e in range(E):
                if e not in reps:
                    load_reps(e)
                g1r, be1r, g2r = reps[e]
                if e not in bves:
                    load_bve(e)
                for c in range(KC):
                    if (e, c) not in w1tiles:
                        load_w_chunk(e, c)
                w1t = [w1tiles[(e, c)] for c in range(KC)]
                w2t = [w2tiles[(e, c)] for c in range(KC)]
                bve = bves[e]

                for tt in range(TT):
                    if e + 1 < E and TT - PREFETCH - 1 <= tt < TT - 1:
                        pc = tt - (TT - PREFETCH - 1)
                        if (e + 1, pc) not in w1tiles:
                            load_w_chunk(e + 1, pc)
                    if e + 1 < E and tt == TT - 2 and (e + 1) not in reps:
                        load_reps(e + 1)
                    if e + 1 < E and tt == TT - 1 and (e + 1) not in bves:
                        load_bve(e + 1)
                    # --- z = x @ W1 + b1 ---
                    z = zp.tile([P, D], F32, tag="z")
                    for c in range(KC):
                        for n in range(NCH):
                            nc.tensor.matmul(
                                z[:, ds(n * 512, 512)],
                                xt_sb[:, c, ts(tt, P)],
                                w1t[c][:, ds(n * 512, 512)],
                                start=(c == 0),
                                stop=False,
                            )
                    for n in range(NCH):
                        nc.tensor.matmul(
                            z[:, ds(n * 512, 512)],
                            ones_bf,
                            bve[:, 0, ds(n * 512, 512)],
                            start=False,
                            stop=True,
                        )
                    # --- LN1 stats ---
                    st1 = statp.tile([P, 2, 6], F32, tag="st1")
                    nc.vector.bn_stats(out=st1[:, 0, :], in_=z[:, 0:512])
                    nc.vector.bn_stats(out=st1[:, 1, :], in_=z[:, 512:1024])
                    mv1 = statp.tile([P, 2], F32, tag="mv1")
                    nc.vector.bn_aggr(out=mv1, in_=st1)
                    sd1 = statp.tile([P, 1], F32, tag="sd1")
                    nc.scalar.activation(
                        out=sd1, in_=mv1[:, 1:2], func=AF.Sqrt, bias=eps_sb
                    )
                    rs1 = statp.tile([P, 1], F32, tag="rs1")
                    nc.vector.reciprocal(out=rs1, in_=sd1)
                    nmr1 = statp.tile([P, 1], F32, tag="nmr1")
                    nc.vector.tensor_scalar(
                        out=nmr1,
                        in0=mv1[:, 0:1],
                        scalar1=rs1,
                        scalar2=-1.0,
                        op0=ALU.mult,
                        op1=ALU.mult,
                    )
                    # --- u = relu((z - m)*rstd*g1 + be1) ---
                    n1 = workp.tile([P, D], F32, tag="n1")
                    nc.scalar.activation(
                        out=n1, in_=z, func=AF.Identity, bias=nmr1, scale=rs1
                    )
                    nc.vector.tensor_tensor(out=n1, in0=n1, in1=g1r, op=ALU.mult)
                    nc.gpsimd.tensor_tensor(out=n1, in0=n1, in1=be1r, op=ALU.add)
                    u = workp.tile([P, D], BF16, tag="u")
                    nc.scalar.activation(out=u, in_=n1, func=AF.Relu)
                    # --- u^T via PE ---
                    utps = utp.tile([P, D], BF16, tag="utp_bf")
                    for c in range(KC):
                        nc.tensor.transpose(utps[:, ts(c, P)], u[:, ts(c, P)], id_bf16)
                    uT = workp.tile([P, KC, P], BF16, tag="uT")
                    utv = utps.rearrange("p (c q) -> p c q", c=KC)
                    nc.scalar.copy(out=uT[:, 0 : KC // 2, :], in_=utv[:, 0 : KC // 2, :])
                    nc.vector.tensor_copy(
                        out=uT[:, KC // 2 :, :], in_=utv[:, KC // 2 :, :]
                    )
                    # --- z2 = u @ W2 + b2 ---
                    z2 = z2p.tile([P, D], F32, tag="z2")
                    for c in range(KC):
                        for n in range(NCH):
                            nc.tensor.matmul(
                                z2[:, ds(n * 512, 512)],
                                uT[:, c, :],
                                w2t[c][:, ds(n * 512, 512)],
                                start=(c == 0),
                                stop=False,
                            )
                    for n in range(NCH):
                        nc.tensor.matmul(
                            z2[:, ds(n * 512, 512)],
                            ones_bf,
                            bve[:, 1, ds(n * 512, 512)],
                            start=False,
                            stop=True,
                        )
                    # --- LN2 stats ---
                    st2 = statp.tile([P, 2, 6], F32, tag="st2")
                    nc.vector.bn_stats(out=st2[:, 0, :], in_=z2[:, 0:512])
                    nc.vector.bn_stats(out=st2[:, 1, :], in_=z2[:, 512:1024])
                    mv2 = statp.tile([P, 2], F32, tag="mv2")
                    nc.vector.bn_aggr(out=mv2, in_=st2)
                    sd2 = statp.tile([P, 1], F32, tag="sd2")
                    nc.scalar.activation(
                        out=sd2, in_=mv2[:, 1:2], func=AF.Sqrt, bias=eps_sb
                    )
                    rs2 = statp.tile([P, 1], F32, tag="rs2")
                    nc.vector.reciprocal(out=rs2, in_=sd2)
                    rw = statp.tile([P, 1], F32, tag="rw")
                    nc.vector.tensor_scalar_mul(
                        out=rw, in0=rs2, scalar1=w_sb[:, tt, e : e + 1]
                    )
                    nmr2 = statp.tile([P, 1], F32, tag="nmr2")
                    nc.vector.tensor_scalar(
                        out=nmr2,
                        in0=mv2[:, 0:1],
                        scalar1=rw,
                        scalar2=-1.0,
                        op0=ALU.mult,
                        op1=ALU.mult,
                    )
                    # --- y_e = (z2 - m2)*rstd2*w_e*g2 ; acc += y_e ---
                    n2 = workp.tile([P, D], F32, tag="n2")
                    nc.scalar.activation(
                        out=n2, in_=z2, func=AF.Identity, bias=nmr2, scale=rw
                    )
                    nc.vector.tensor_tensor(out=n2, in0=n2, in1=g2r, op=ALU.mult)
                    if e == 0:
                        xres = xinp.tile([P, D], F32, tag="xin")
                        nc.sync.dma_start(out=xres, in_=x_d[ts(tt, P), :])
                        acc[tt] = accp.tile([P, D], F32, tag="acc", name=f"acc_{tt}")
                        nc.gpsimd.tensor_tensor(
                            out=acc[tt], in0=n2, in1=xres, op=ALU.add
                        )
                    else:
                        nc.gpsimd.tensor_tensor(
                            out=acc[tt], in0=n2, in1=acc[tt], op=ALU.add
                        )
            utp_ctx.__exit__(None, None, None)
            z2p_ctx.__exit__(None, None, None)
            zp_ctx.__exit__(None, None, None)
            cpp_ctx = tc.tile_pool(name="cpp", bufs=2, space="PSUM")
            cpp = cpp_ctx.__enter__()

            # ---- finalize phase: out = acc + w @ be2 ----
            for tt in range(TT):
                outt = workp.tile([P, D], F32, tag="n1")
                for n in range(NCH):
                    cps = cpp.tile([P, 512], F32, tag="cp", name=f"cp_{tt}_{n}")
                    nc.tensor.matmul(
                        cps,
                        wT_sb[:, tt, :],
                        be2_sb[:, ds(n * 512, 512)],
                        start=True,
                        stop=True,
                    )
                    nc.vector.tensor_tensor(
                        out=outt[:, ds(n * 512, 512)],
                        in0=cps,
                        in1=acc[tt][:, ds(n * 512, 512)],
                        op=ALU.add,
                    )
                nc.sync.dma_start(out=out_d[ts(tt, P), :], in_=outt)

            cpp_ctx.__exit__(None, None, None)

    nc.compile()
    return nc


_nc_cache = {}
_nc_lock = threading.Lock()


def _get_nc(T, num_devices):
    key = (T, num_devices)
    with _nc_lock:
        if key not in _nc_cache:
            _nc_cache[key] = build_moe_nc(T, num_devices)
        return _nc_cache[key]


def kernel(**inputs) -> np.ndarray:
    from concourse.bass_utils import run_bass_kernel_spmd

    x = np.ascontiguousarray(np.asarray(inputs["x"], dtype=np.float32))
    B, N, Dd = x.shape
    assert Dd == D and B == N_CORES, (B, N, Dd)
    weights = {
        k: np.ascontiguousarray(np.asarray(inputs[k], dtype=np.float32))
        for k in (
            "gate_W",
            "gate_b",
            "W1",
            "b1",
            "g1",
            "be1",
            "W2",
            "b2",
            "g2",
            "be2",
        )
    }
    nc = _get_nc(N, N_CORES)
    in_maps = [dict(weights, x=x[i]) for i in range(N_CORES)]
    res = run_bass_kernel_spmd(nc, in_maps, core_ids=list(range(N_CORES)))
    out = np.stack([r["out"] for r in res.results], axis=0)
    return out.astype(np.float32)



# revision 4
# speedup vs baseline: 1.8067x; 1.8067x over previous
"""MoE (E=4 experts, top-2 routing) forward on 8 Trainium2 NeuronCores.

Data-parallel over tokens: core i processes batch row i (2048 tokens);
expert weights replicated per core.

Fast path (build_moe_routed_nc): actually routes tokens -- computes the
top-2 assignment on device, compacts tokens into per-expert slot tiles via
prefix-sum/one-hot matmuls and indirect DMA gathers, runs each expert's
FFN only on its assigned tokens (capacity 1152 = 9 tiles per expert), and
recombines with gathered expert outputs. This does ~half the matmul work
of the dense approach. It assumes the (verified-on-host) specializations:
gate_b=0, b1=b2=0, g1=g2=1, be1=be2=0, and per-(core,expert) assignment
counts <= 1152.

Fallback (build_moe_nc): dense all-experts compute, correct for any
inputs; used when the fast-path preconditions do not hold.
"""

import threading

import numpy as np

import concourse.bass as bass
import concourse.mybir as mybir
import concourse.tile as tile
from concourse import bacc
from concourse.bass import ds, ts
from concourse.masks import make_identity, make_upper_triangular

F32 = mybir.dt.float32
BF16 = mybir.dt.bfloat16
F16 = mybir.dt.float16
I32 = mybir.dt.int32
AF = mybir.ActivationFunctionType
ALU = mybir.AluOpType
AX = mybir.AxisListType

P = 128
D = 1024
E = 4
KC = D // P
NCH = D // 512
LN_EPS = 1e-5
CAP = 1152
TAU = CAP // P  # 9 slot-tiles per expert
N_CORES = 8


def _row1(ap):
    """Lift an AP to have a leading length-1 (partition) dim."""
    return bass.AP(tensor=ap.tensor, offset=ap.offset, ap=[[0, 1]] + list(ap.ap))


def _bcast_rows(ap_row, p=P):
    """Broadcast a [1, N]-ish DRAM AP across p partitions (step-0 partition dim)."""
    inner = [list(d) for d in ap_row.ap if d[1] != 1]
    return bass.AP(tensor=ap_row.tensor, offset=ap_row.offset, ap=[[0, p]] + inner)


def build_moe_nc(T=2048, num_devices=N_CORES):
    TT = T // P
    nc = bacc.Bacc(
        "TRN2", target_bir_lowering=False, debug=False, num_devices=num_devices
    )

    x_d = nc.dram_tensor("x", [T, D], F32, kind="ExternalInput")
    gw_d = nc.dram_tensor("gate_W", [D, E], F32, kind="ExternalInput")
    gb_d = nc.dram_tensor("gate_b", [E], F32, kind="ExternalInput")
    w1_d = nc.dram_tensor("W1", [E, D, D], F32, kind="ExternalInput")
    b1_d = nc.dram_tensor("b1", [E, D], F32, kind="ExternalInput")
    g1_d = nc.dram_tensor("g1", [E, D], F32, kind="ExternalInput")
    be1_d = nc.dram_tensor("be1", [E, D], F32, kind="ExternalInput")
    w2_d = nc.dram_tensor("W2", [E, D, D], F32, kind="ExternalInput")
    b2_d = nc.dram_tensor("b2", [E, D], F32, kind="ExternalInput")
    g2_d = nc.dram_tensor("g2", [E, D], F32, kind="ExternalInput")
    be2_d = nc.dram_tensor("be2", [E, D], F32, kind="ExternalInput")
    out_d = nc.dram_tensor("out", [T, D], F32, kind="ExternalOutput")

    with tile.TileContext(nc) as tc:
        with (
            tc.tile_pool(name="const", bufs=1) as const,
            tc.tile_pool(name="w1p", bufs=12) as w1p,
            tc.tile_pool(name="w2p", bufs=12) as w2p,
            tc.tile_pool(name="repp", bufs=2) as repp,
            tc.tile_pool(name="bvep", bufs=2) as bvep,
            tc.tile_pool(name="accp", bufs=TT) as accp,
            tc.tile_pool(name="workp", bufs=2) as workp,
            tc.tile_pool(name="xinp", bufs=2) as xinp,
            tc.tile_pool(name="statp", bufs=3) as statp,
            tc.tile_pool(name="gstp", bufs=1) as gstp,
        ):
            # ---- constants ----
            id_f32 = const.tile([P, P], F32)
            make_identity(nc, id_f32)
            id_bf16 = const.tile([P, P], BF16)
            make_identity(nc, id_bf16)
            ones_bf = const.tile([1, P], BF16)
            nc.vector.memset(ones_bf, 1.0)
            ones_f32 = const.tile([1, P], F32)
            nc.vector.memset(ones_f32, 1.0)
            eps_sb = const.tile([P, 1], F32)
            nc.vector.memset(eps_sb, LN_EPS)

            gw_sb = const.tile([P, KC, E], F32)
            nc.sync.dma_start(out=gw_sb, in_=gw_d.rearrange("(c p) e -> p c e", p=P))
            gb_sb = const.tile([1, E], F32)
            nc.sync.dma_start(out=gb_sb, in_=_row1(gb_d[:]))

            be2_sb = const.tile([E, D], BF16)
            nc.gpsimd.dma_start(out=be2_sb, in_=be2_d[:, :])  # casting dma

            xt_sb = const.tile([P, KC, T], BF16)  # x^T, matmul lhsT layout
            scores_sb = const.tile([P, TT, E], F32)
            w_sb = const.tile([P, TT, E], F32)
            wT_sb = const.tile([E, TT, P], BF16)

            w1tiles = {}
            w2tiles = {}
            bves = {}

            def load_w_chunk(e, c):
                t1w = w1p.tile([P, D], BF16, tag="w1", name=f"w1_{e}_{c}")
                nc.gpsimd.dma_start(out=t1w, in_=w1_d[e, ts(c, P), :])
                w1tiles[(e, c)] = t1w
                t2w = w2p.tile([P, D], BF16, tag="w2", name=f"w2_{e}_{c}")
                nc.gpsimd.dma_start(out=t2w, in_=w2_d[e, ts(c, P), :])
                w2tiles[(e, c)] = t2w

            for _c in range(KC):
                load_w_chunk(0, _c)

            # ---- prologue: transpose x, gate scores ----
            pre_ctx = tc.tile_pool(name="prep", bufs=2, space="PSUM")
            prep = pre_ctx.__enter__()
            for tt in range(TT):
                xin = xinp.tile([P, D], F32, tag="xin")
                nc.sync.dma_start(out=xin, in_=x_d[ts(tt, P), :])
                tp = prep.tile([P, D], F32, tag="tp")
                for c in range(KC):
                    nc.tensor.transpose(tp[:, ts(c, P)], xin[:, ts(c, P)], id_f32)
                xtg = workp.tile([P, D], F32, tag="n1")
                nc.scalar.copy(out=xtg, in_=tp)
                nc.vector.tensor_copy(
                    out=xt_sb[:, :, ts(tt, P)],
                    in_=tp.rearrange("p (c q) -> p c q", c=KC),
                )
                gps = prep.tile([P, E], F32, tag="gate")
                for c in range(KC):
                    nc.tensor.matmul(
                        gps,
                        xtg[:, ts(c, P)],
                        gw_sb[:, c, :],
                        start=(c == 0),
                        stop=False,
                    )
                nc.tensor.matmul(gps, ones_f32, gb_sb, start=False, stop=True)
                nc.scalar.copy(out=scores_sb[:, tt, :], in_=gps)

            # ---- top-2 softmax over the E=4 scores ----
            s3 = scores_sb  # [P, TT, E]
            m1 = gstp.tile([P, TT], F32, tag="m1")
            nc.vector.tensor_reduce(out=m1, in_=s3, axis=AX.X, op=ALU.max)
            m1b = m1.broadcast_to((P, TT, E))
            eqt = gstp.tile([P, TT, E], F32, tag="eqt")
            nc.vector.tensor_tensor(out=eqt, in0=s3, in1=m1b, op=ALU.is_equal)
            smt = gstp.tile([P, TT, E], F32, tag="smt")
            nc.vector.scalar_tensor_tensor(
                out=smt, in0=eqt, scalar=-1e30, in1=s3, op0=ALU.mult, op1=ALU.add
            )
            m2 = gstp.tile([P, TT], F32, tag="m2")
            nc.vector.tensor_reduce(out=m2, in_=smt, axis=AX.X, op=ALU.max)
            m2b = m2.broadcast_to((P, TT, E))
            ind = gstp.tile([P, TT, E], F32, tag="ind")
            nc.vector.tensor_tensor(out=ind, in0=s3, in1=m2b, op=ALU.is_ge)
            dd = gstp.tile([P, TT, E], F32, tag="dd")
            nc.vector.tensor_tensor(out=dd, in0=s3, in1=m1b, op=ALU.subtract)
            ex = gstp.tile([P, TT, E], F32, tag="ex")
            nc.scalar.activation(out=ex, in_=dd, func=AF.Exp)
            en = gstp.tile([P, TT, E], F32, tag="en")
            nc.vector.tensor_tensor(out=en, in0=ex, in1=ind, op=ALU.mult)
            zs = gstp.tile([P, TT], F32, tag="zs")
            nc.vector.tensor_reduce(out=zs, in_=en, axis=AX.X, op=ALU.add)
            rz = gstp.tile([P, TT], F32, tag="rz")
            nc.vector.reciprocal(out=rz, in_=zs)
            rzb = rz.broadcast_to((P, TT, E))
            nc.vector.tensor_tensor(out=w_sb, in0=en, in1=rzb, op=ALU.mult)
            for tt in range(TT):
                wtp = prep.tile([E, P], F32, tag="gate")
                nc.tensor.transpose(wtp, w_sb[:, tt, :], id_f32)
                nc.scalar.copy(out=wT_sb[:, tt, :], in_=wtp)

            pre_ctx.__exit__(None, None, None)
            zp_ctx = tc.tile_pool(name="zp", bufs=2, space="PSUM")
            zp = zp_ctx.__enter__()
            z2p_ctx = tc.tile_pool(name="z2p", bufs=1, space="PSUM")
            z2p = z2p_ctx.__enter__()
            utp_ctx = tc.tile_pool(name="utp", bufs=2, space="PSUM")
            utp = utp_ctx.__enter__()

            # ---- dense expert loop ----
            acc = {}

            def load_bve(e):
                bve = bvep.tile([1, 2, D], BF16, tag="bve", name=f"bve_{e}")
                nc.gpsimd.dma_start(out=bve[:, 0, :], in_=_row1(b1_d[e, :]))
                nc.gpsimd.dma_start(out=bve[:, 1, :], in_=_row1(b2_d[e, :]))
                bves[e] = bve

            reps = {}

            def load_reps(e):
                g1r = repp.tile([P, D], BF16, tag="g1r", name=f"g1r_{e}")
                nc.gpsimd.dma_start(out=g1r, in_=_bcast_rows(g1_d[e : e + 1, :]))
                be1r = repp.tile([P, D], BF16, tag="be1r", name=f"be1r_{e}")
                nc.gpsimd.dma_start(out=be1r, in_=_bcast_rows(be1_d[e : e + 1, :]))
                g2r = repp.tile([P, D], BF16, tag="g2r", name=f"g2r_{e}")
                nc.gpsimd.dma_start(out=g2r, in_=_bcast_rows(g2_d[e : e + 1, :]))
                reps[e] = (g1r, be1r, g2r)

            PREFETCH = 4  # chunks of expert e+1 issued inside expert e's loop
            for e in range(E):
                if e not in reps:
                    load_reps(e)
                g1r, be1r, g2r = reps[e]
                if e not in bves:
                    load_bve(e)
                for c in range(KC):
                    if (e, c) not in w1tiles:
                        load_w_chunk(e, c)
                w1t = [w1tiles[(e, c)] for c in range(KC)]
                w2t = [w2tiles[(e, c)] for c in range(KC)]
                bve = bves[e]

                for tt in range(TT):
                    if e + 1 < E and TT - PREFETCH - 1 <= tt < TT - 1:
                        pc = tt - (TT - PREFETCH - 1)
                        if (e + 1, pc) not in w1tiles:
                            load_w_chunk(e + 1, pc)
                    if e + 1 < E and tt == TT - 2 and (e + 1) not in reps:
                        load_reps(e + 1)
                    if e + 1 < E and tt == TT - 1 and (e + 1) not in bves:
                        load_bve(e + 1)
                    # --- z = x @ W1 + b1 ---
                    z = zp.tile([P, D], F32, tag="z")
                    for c in range(KC):
                        for n in range(NCH):
                            nc.tensor.matmul(
                                z[:, ds(n * 512, 512)],
                                xt_sb[:, c, ts(tt, P)],
                                w1t[c][:, ds(n * 512, 512)],
                                start=(c == 0),
                                stop=False,
                            )
                    for n in range(NCH):
                        nc.tensor.matmul(
                            z[:, ds(n * 512, 512)],
                            ones_bf,
                            bve[:, 0, ds(n * 512, 512)],
                            start=False,
                            stop=True,
                        )
                    # --- LN1 stats ---
                    st1 = statp.tile([P, 2, 6], F32, tag="st1")
                    nc.vector.bn_stats(out=st1[:, 0, :], in_=z[:, 0:512])
                    nc.vector.bn_stats(out=st1[:, 1, :], in_=z[:, 512:1024])
                    mv1 = statp.tile([P, 2], F32, tag="mv1")
                    nc.vector.bn_aggr(out=mv1, in_=st1)
                    sd1 = statp.tile([P, 1], F32, tag="sd1")
                    nc.scalar.activation(
                        out=sd1, in_=mv1[:, 1:2], func=AF.Sqrt, bias=eps_sb
                    )
                    rs1 = statp.tile([P, 1], F32, tag="rs1")
                    nc.vector.reciprocal(out=rs1, in_=sd1)
                    nmr1 = statp.tile([P, 1], F32, tag="nmr1")
                    nc.vector.tensor_scalar(
                        out=nmr1,
                        in0=mv1[:, 0:1],
                        scalar1=rs1,
                        scalar2=-1.0,
                        op0=ALU.mult,
                        op1=ALU.mult,
                    )
                    # --- u = relu((z - m)*rstd*g1 + be1) ---
                    n1 = workp.tile([P, D], F32, tag="n1")
                    nc.scalar.activation(
                        out=n1, in_=z, func=AF.Identity, bias=nmr1, scale=rs1
                    )
                    nc.vector.tensor_tensor(out=n1, in0=n1, in1=g1r, op=ALU.mult)
                    nc.gpsimd.tensor_tensor(out=n1, in0=n1, in1=be1r, op=ALU.add)
                    u = workp.tile([P, D], BF16, tag="u")
                    nc.scalar.activation(out=u, in_=n1, func=AF.Relu)
                    # --- u^T via PE ---
                    utps = utp.tile([P, D], BF16, tag="utp_bf")
                    for c in range(KC):
                        nc.tensor.transpose(utps[:, ts(c, P)], u[:, ts(c, P)], id_bf16)
                    uT = workp.tile([P, KC, P], BF16, tag="uT")
                    utv = utps.rearrange("p (c q) -> p c q", c=KC)
                    nc.scalar.copy(out=uT[:, 0 : KC // 2, :], in_=utv[:, 0 : KC // 2, :])
                    nc.vector.tensor_copy(
                        out=uT[:, KC // 2 :, :], in_=utv[:, KC // 2 :, :]
                    )
                    # --- z2 = u @ W2 + b2 ---
                    z2 = z2p.tile([P, D], F32, tag="z2")
                    for c in range(KC):
                        for n in range(NCH):
                            nc.tensor.matmul(
                                z2[:, ds(n * 512, 512)],
                                uT[:, c, :],
                                w2t[c][:, ds(n * 512, 512)],
                                start=(c == 0),
                                stop=False,
                            )
                    for n in range(NCH):
                        nc.tensor.matmul(
                            z2[:, ds(n * 512, 512)],
                            ones_bf,
                            bve[:, 1, ds(n * 512, 512)],
                            start=False,
                            stop=True,
                        )
                    # --- LN2 stats ---
                    st2 = statp.tile([P, 2, 6], F32, tag="st2")
                    nc.vector.bn_stats(out=st2[:, 0, :], in_=z2[:, 0:512])
                    nc.vector.bn_stats(out=st2[:, 1, :], in_=z2[:, 512:1024])
                    mv2 = statp.tile([P, 2], F32, tag="mv2")
                    nc.vector.bn_aggr(out=mv2, in_=st2)
                    sd2 = statp.tile([P, 1], F32, tag="sd2")
                    nc.scalar.activation(
                        out=sd2, in_=mv2[:, 1:2], func=AF.Sqrt, bias=eps_sb
                    )
                    rs2 = statp.tile([P, 1], F32, tag="rs2")
                    nc.vector.reciprocal(out=rs2, in_=sd2)
                    rw = statp.tile([P, 1], F32, tag="rw")
                    nc.vector.tensor_scalar_mul(
                        out=rw, in0=rs2, scalar1=w_sb[:, tt, e : e + 1]
                    )
                    nmr2 = statp.tile([P, 1], F32, tag="nmr2")
                    nc.vector.tensor_scalar(
                        out=nmr2,
                        in0=mv2[:, 0:1],
                        scalar1=rw,
                        scalar2=-1.0,
                        op0=ALU.mult,
                        op1=ALU.mult,
                    )
                    # --- y_e = (z2 - m2)*rstd2*w_e*g2 ; acc += y_e ---
                    n2 = workp.tile([P, D], F32, tag="n2")
                    nc.scalar.activation(
                        out=n2, in_=z2, func=AF.Identity, bias=nmr2, scale=rw
                    )
                    nc.vector.tensor_tensor(out=n2, in0=n2, in1=g2r, op=ALU.mult)
                    if e == 0:
                        xres = xinp.tile([P, D], F32, tag="xin")
                        nc.sync.dma_start(out=xres, in_=x_d[ts(tt, P), :])
                        acc[tt] = accp.tile([P, D], F32, tag="acc", name=f"acc_{tt}")
                        nc.gpsimd.tensor_tensor(
                            out=acc[tt], in0=n2, in1=xres, op=ALU.add
                        )
                    else:
                        nc.gpsimd.tensor_tensor(
                            out=acc[tt], in0=n2, in1=acc[tt], op=ALU.add
                        )
            utp_ctx.__exit__(None, None, None)
            z2p_ctx.__exit__(None, None, None)
            zp_ctx.__exit__(None, None, None)
            cpp_ctx = tc.tile_pool(name="cpp", bufs=2, space="PSUM")
            cpp = cpp_ctx.__enter__()

            # ---- finalize phase: out = acc + w @ be2 ----
            for tt in range(TT):
                outt = workp.tile([P, D], F32, tag="n1")
                for n in range(NCH):
                    cps = cpp.tile([P, 512], F32, tag="cp", name=f"cp_{tt}_{n}")
                    nc.tensor.matmul(
                        cps,
                        wT_sb[:, tt, :],
                        be2_sb[:, ds(n * 512, 512)],
                        start=True,
                        stop=True,
                    )
                    nc.vector.tensor_tensor(
                        out=outt[:, ds(n * 512, 512)],
                        in0=cps,
                        in1=acc[tt][:, ds(n * 512, 512)],
                        op=ALU.add,
                    )
                nc.sync.dma_start(out=out_d[ts(tt, P), :], in_=outt)

            cpp_ctx.__exit__(None, None, None)

    nc.compile()
    return nc




def build_moe_routed_nc(T=2048, num_devices=N_CORES):
    TT = T // P  # 16 token tiles
    nc = bacc.Bacc(
        "TRN2", target_bir_lowering=False, debug=False, num_devices=num_devices
    )

    x_d = nc.dram_tensor("x", [T, D], F32, kind="ExternalInput")
    gw_d = nc.dram_tensor("gate_W", [D, E], F32, kind="ExternalInput")
    w1_d = nc.dram_tensor("W1", [E, D, D], F32, kind="ExternalInput")
    w2_d = nc.dram_tensor("W2", [E, D, D], F32, kind="ExternalInput")
    out_d = nc.dram_tensor("out", [T, D], F32, kind="ExternalOutput")
    y_d = nc.dram_tensor("y_scratch", [E * CAP, D], BF16)

    with tile.TileContext(nc) as tc:
        with (
            tc.tile_pool(name="const", bufs=1) as const,
            tc.tile_pool(name="w1p", bufs=2) as w1p,
            tc.tile_pool(name="w2p", bufs=2) as w2p,
            tc.tile_pool(name="gstp", bufs=1) as gstp,
            tc.tile_pool(name="xgp", bufs=3) as xgp,
            tc.tile_pool(name="workp", bufs=2) as workp,
            tc.tile_pool(name="statp", bufs=3) as statp,
        ):
            # ---------------- constants ----------------
            id_f32 = const.tile([P, P], F32)
            make_identity(nc, id_f32)
            id_bf16 = const.tile([P, P], BF16)
            make_identity(nc, id_bf16)
            ones_mat = const.tile([P, P], F32)
            nc.vector.memset(ones_mat, 1.0)
            ltri = const.tile([P, P], F32)  # ltri[p,q] = 1 iff p < q
            make_upper_triangular(nc, ltri, val=1.0, diag=False)
            eps_sb = const.tile([P, 1], F32)
            nc.vector.memset(eps_sb, LN_EPS)

            gw_sb = const.tile([P, KC, E], F32)
            nc.sync.dma_start(out=gw_sb, in_=gw_d.rearrange("(c p) e -> p c e", p=P))

            # iotas
            iota_tok_i = const.tile([P, TT], I32)  # p + 128*tt
            nc.gpsimd.iota(iota_tok_i, pattern=[[P, TT]], base=0, channel_multiplier=1)
            ids_f16 = const.tile([P, TT], F16)
            nc.vector.tensor_copy(out=ids_f16, in_=iota_tok_i)
            iota128_i = const.tile([P, P], I32)  # 0..127 along free, all parts
            nc.gpsimd.iota(iota128_i, pattern=[[1, P]], base=0, channel_multiplier=0)
            iota128_f16 = const.tile([P, P], F16)
            nc.vector.tensor_copy(out=iota128_f16, in_=iota128_i)
            iota4_i = const.tile([P, P, E], I32)  # v[p, q, e] = q
            nc.gpsimd.iota(iota4_i, pattern=[[1, P], [0, E]], base=0, channel_multiplier=0)
            iota4_f16 = const.tile([P, P, E], F16)
            nc.vector.tensor_copy(out=iota4_f16, in_=iota4_i)
            thr_i = const.tile([P, TAU], I32)  # 128,256,...,1152
            nc.gpsimd.iota(thr_i, pattern=[[P, TAU]], base=P, channel_multiplier=0)
            thr_f16 = const.tile([P, TAU], F16)
            nc.vector.tensor_copy(out=thr_f16, in_=thr_i)
            tau_i = const.tile([P, TAU], I32)  # 0..8
            nc.gpsimd.iota(tau_i, pattern=[[1, TAU]], base=0, channel_multiplier=0)
            tau_f16 = const.tile([P, TAU], F16)
            nc.vector.tensor_copy(out=tau_f16, in_=tau_i)
            ebase_i = const.tile([P, E], I32)  # 0,1152,2304,3456
            nc.gpsimd.iota(ebase_i, pattern=[[CAP, E]], base=0, channel_multiplier=0)
            ebase_f = const.tile([P, E], F32)
            nc.vector.tensor_copy(out=ebase_f, in_=ebase_i)

            x_res = const.tile([P, TT, D], F32)  # resident x (64KB/part)
            scores_sb = const.tile([P, TT, E], F32)

            # ---------------- gating ----------------
            xtp_ctx = tc.tile_pool(name="xtp", bufs=2)
            xtp = xtp_ctx.__enter__()
            for tt in range(TT):
                nc.gpsimd.dma_start(out=x_res[:, tt, :], in_=x_d[ts(tt, P), :])
            g_ctx = tc.tile_pool(name="gpsum", bufs=2, space="PSUM")
            gp = g_ctx.__enter__()

            xtgs = {}

            def gate_mm(tt):
                gps = gp.tile([P, E], F32, tag="gate")
                xtg = xtgs.pop(tt)
                for c in range(KC):
                    nc.tensor.matmul(
                        gps,
                        xtg[:, ts(c, P)],
                        gw_sb[:, c, :],
                        start=(c == 0),
                        stop=(c == KC - 1),
                    )
                nc.scalar.copy(out=scores_sb[:, tt, :], in_=gps)

            for tt in range(TT):
                tp = gp.tile([P, D], F32, tag="tp")
                for c in range(KC):
                    nc.tensor.transpose(
                        tp[:, ts(c, P)], x_res[:, tt, ts(c, P)], id_f32
                    )
                xtg = xtp.tile([P, D], F32, tag="xt")
                if tt % 2 == 0:
                    nc.scalar.copy(out=xtg, in_=tp)
                else:
                    nc.vector.tensor_copy(out=xtg, in_=tp)
                xtgs[tt] = xtg
                if tt > 0:
                    gate_mm(tt - 1)
            gate_mm(TT - 1)
            g_ctx.__exit__(None, None, None)
            xtp_ctx.__exit__(None, None, None)

            # expert-0 weights: one monolithic casting DMA per matrix
            w1tiles = {}
            w2tiles = {}

            WSPLIT = 4  # pieces per matrix load: avoid hogging DMA engines

            def load_w1(e):
                t1w = w1p.tile([P, KC, D], BF16, tag="w1", name=f"w1_{e}")
                wsrc = w1_d[e].rearrange("(c p) d -> p c d", p=P)
                for s in range(WSPLIT):
                    cs = KC // WSPLIT
                    nc.gpsimd.dma_start(
                        out=t1w[:, s * cs : (s + 1) * cs, :],
                        in_=wsrc[:, s * cs : (s + 1) * cs, :],
                    )
                w1tiles[e] = t1w

            def load_w2(e):
                t2w = w2p.tile([P, KC, D], BF16, tag="w2", name=f"w2_{e}")
                wsrc = w2_d[e].rearrange("(c p) d -> p c d", p=P)
                for s in range(WSPLIT):
                    cs = KC // WSPLIT
                    nc.gpsimd.dma_start(
                        out=t2w[:, s * cs : (s + 1) * cs, :],
                        in_=wsrc[:, s * cs : (s + 1) * cs, :],
                    )
                w2tiles[e] = t2w

            # ---------------- top-2 softmax ----------------
            s3 = scores_sb
            m1 = gstp.tile([P, TT], F32, tag="m1")
            nc.vector.tensor_reduce(out=m1, in_=s3, axis=AX.X, op=ALU.max)
            m1b = m1.broadcast_to((P, TT, E))
            eqt = gstp.tile([P, TT, E], F32, tag="eqt")
            nc.vector.tensor_tensor(out=eqt, in0=s3, in1=m1b, op=ALU.is_equal)
            smt = gstp.tile([P, TT, E], F32, tag="smt")
            nc.vector.scalar_tensor_tensor(
                out=smt, in0=eqt, scalar=-1e30, in1=s3, op0=ALU.mult, op1=ALU.add
            )
            m2 = gstp.tile([P, TT], F32, tag="m2")
            nc.vector.tensor_reduce(out=m2, in_=smt, axis=AX.X, op=ALU.max)
            m2b = m2.broadcast_to((P, TT, E))
            ind = gstp.tile([P, TT, E], F32, tag="ind")
            nc.vector.tensor_tensor(out=ind, in0=s3, in1=m2b, op=ALU.is_ge)
            dd = gstp.tile([P, TT, E], F32, tag="dd")
            nc.vector.tensor_tensor(out=dd, in0=s3, in1=m1b, op=ALU.subtract)
            ex = gstp.tile([P, TT, E], F32, tag="ex")
            nc.scalar.activation(out=ex, in_=dd, func=AF.Exp)
            en = gstp.tile([P, TT, E], F32, tag="en")
            nc.vector.tensor_tensor(out=en, in0=ex, in1=ind, op=ALU.mult)
            zs = gstp.tile([P, TT], F32, tag="zs")
            nc.vector.tensor_reduce(out=zs, in_=en, axis=AX.X, op=ALU.add)
            rz = gstp.tile([P, TT], F32, tag="rz")
            nc.vector.reciprocal(out=rz, in_=zs)
            rzb = rz.broadcast_to((P, TT, E))
            w_sb = gstp.tile([P, TT, E], F32, tag="w")
            nc.vector.tensor_tensor(out=w_sb, in0=en, in1=rzb, op=ALU.mult)
            # per-token combine weights interleaved [P, TT, {A,B}]
            mB = gstp.tile([P, TT, E], F32, tag="mB")
            nc.vector.tensor_tensor(out=mB, in0=ind, in1=eqt, op=ALU.subtract)
            wAB = gstp.tile([P, TT, 2], F32, tag="wAB")
            wAe = gstp.tile([P, TT, E], F32, tag="wAe")
            nc.vector.tensor_tensor(out=wAe, in0=w_sb, in1=eqt, op=ALU.mult)
            nc.vector.tensor_reduce(out=wAB[:, :, 0:1], in_=wAe, axis=AX.X, op=ALU.add)
            wBe = gstp.tile([P, TT, E], F32, tag="wBe")
            nc.vector.tensor_tensor(out=wBe, in0=w_sb, in1=mB, op=ALU.mult)
            nc.vector.tensor_reduce(out=wAB[:, :, 1:2], in_=wBe, axis=AX.X, op=ALU.add)

            load_w1(0)
            load_w2(0)

            # ---------------- slot assignment (prefix sums) ----------------
            l_ctx = tc.tile_pool(name="lpsum", bufs=1, space="PSUM")
            lp = l_ctx.__enter__()
            posp = lp.tile([P, TT, E], F32, tag="posp")
            cntp = lp.tile([P, E, TT], F32, tag="cntp")
            for tt in range(TT):
                nc.tensor.matmul(
                    posp[:, tt, :], ltri, ind[:, tt, :], start=True, stop=True
                )
                nc.tensor.matmul(
                    cntp[:, :, tt], ones_mat, ind[:, tt, :], start=True, stop=True
                )
            # exclusive prefix over tt per expert (Hillis-Steele, ping-pong)
            offa = gstp.tile([P, E, TT], F32, tag="offa")
            nc.vector.memset(offa[:, :, 0:1], 0.0)
            nc.vector.tensor_copy(out=offa[:, :, 1:], in_=cntp[:, :, : TT - 1])
            offb = gstp.tile([P, E, TT], F32, tag="offb")
            cur, nxt = offa, offb
            k = 1
            while k < TT:
                nc.vector.tensor_copy(out=nxt[:, :, :k], in_=cur[:, :, :k])
                nc.vector.tensor_tensor(
                    out=nxt[:, :, k:],
                    in0=cur[:, :, k:],
                    in1=cur[:, :, : TT - k],
                    op=ALU.add,
                )
                cur, nxt = nxt, cur
                k *= 2
            off = cur  # [P, E, TT] exclusive offsets, replicated over partitions
            posg = gstp.tile([P, TT, E], F32, tag="posg")
            off_tte = bass.AP(
                tensor=off.tensor,
                offset=off.offset,
                ap=[list(off.ap[0]), [1, TT], [TT, E]],
            )
            nc.vector.tensor_tensor(out=posg, in0=posp, in1=off_tte, op=ALU.add)
            # combine gather indices: s = ebase[e] + posg, reduced over top1/top2
            sbase = gstp.tile([P, TT, E], F32, tag="sbase")
            ebase_b = bass.AP(
                tensor=ebase_f.tensor,
                offset=ebase_f.offset,
                ap=[list(ebase_f.ap[0]), [0, TT], [1, E]],
            )
            nc.vector.tensor_tensor(out=sbase, in0=posg, in1=ebase_b, op=ALU.add)
            sAB_f = gstp.tile([P, TT, 2], F32, tag="sABf")
            sAe = gstp.tile([P, TT, E], F32, tag="sAe")
            nc.vector.tensor_tensor(out=sAe, in0=sbase, in1=eqt, op=ALU.mult)
            nc.vector.tensor_reduce(
                out=sAB_f[:, :, 0:1], in_=sAe, axis=AX.X, op=ALU.add
            )
            sBe = gstp.tile([P, TT, E], F32, tag="sBe")
            nc.vector.tensor_tensor(out=sBe, in0=sbase, in1=mB, op=ALU.mult)
            nc.vector.tensor_reduce(
                out=sAB_f[:, :, 1:2], in_=sBe, axis=AX.X, op=ALU.add
            )
            sAB_i = gstp.tile([P, TT, 2], I32, tag="sABi")
            nc.vector.tensor_copy(out=sAB_i, in_=sAB_f)

            # mask out non-selected (token,expert): pos -> CAP (dropped)
            posm = gstp.tile([P, TT, E], F32, tag="posm")
            pshift = gstp.tile([P, TT, E], F32, tag="pshift")
            nc.vector.tensor_scalar(
                out=pshift,
                in0=posg,
                scalar1=1.0,
                scalar2=-float(CAP),
                op0=ALU.mult,
                op1=ALU.add,
            )
            nc.vector.tensor_tensor(out=posm, in0=pshift, in1=ind, op=ALU.mult)
            nc.vector.tensor_scalar(
                out=posm,
                in0=posm,
                scalar1=1.0,
                scalar2=float(CAP),
                op0=ALU.mult,
                op1=ALU.add,
            )
            pos16 = gstp.tile([P, TT, E], F16, tag="pos16")
            nc.vector.tensor_copy(out=pos16, in_=posm)

            # tau(p) = #{thresholds <= pos} ; r = pos - 128*tau ; band masks
            geX = gstp.tile([P, TT, E, TAU], F16, tag="geX")
            pos_b = pos16.broadcast_to((P, TT, E, TAU))
            thr_b = bass.AP(
                tensor=thr_f16.tensor,
                offset=thr_f16.offset,
                ap=[list(thr_f16.ap[0]), [0, TT], [0, E], [1, TAU]],
            )
            nc.vector.tensor_tensor(out=geX, in0=pos_b, in1=thr_b, op=ALU.is_ge)
            taup = gstp.tile([P, TT, E], F16, tag="taup")
            with nc.allow_low_precision(reason="small exact integers in fp16"):
                nc.vector.tensor_reduce(out=taup, in_=geX, axis=AX.X, op=ALU.add)
            rp = gstp.tile([P, TT, E], F16, tag="rp")
            nc.vector.scalar_tensor_tensor(
                out=rp, in0=taup, scalar=-128.0, in1=pos16, op0=ALU.mult, op1=ALU.add
            )
            band = gstp.tile([P, TT, E, TAU], F16, tag="band")
            tau_b = bass.AP(
                tensor=tau_f16.tensor,
                offset=tau_f16.offset,
                ap=[list(tau_f16.ap[0]), [0, TT], [0, E], [1, TAU]],
            )
            nc.vector.tensor_tensor(
                out=band, in0=taup.broadcast_to((P, TT, E, TAU)), in1=tau_b,
                op=ALU.is_equal,
            )
            idm = gstp.tile([P, TT, E, TAU], F16, tag="idm")
            ids_b = bass.AP(
                tensor=ids_f16.tensor,
                offset=ids_f16.offset,
                ap=[list(ids_f16.ap[0]), [1, TT], [0, E], [0, TAU]],
            )
            nc.vector.tensor_tensor(out=idm, in0=band, in1=ids_b, op=ALU.mult)

            # one-hot scatter matmuls -> compacted token-id lists
            lists_i = gstp.tile([P, E, TAU], I32, tag="listsi")
            lanep_ctx = tc.tile_pool(name="lanep", bufs=2)
            lanep = lanep_ctx.__enter__()
            # full-bank PSUM tiles: each expert's accumulation group gets its
            # own zero region
            listps = [
                lp.tile([P, 512], F32, tag=f"listp{e}", name=f"listp{e}")
                for e in range(E)
            ]
            for tt in range(TT):
                rt4 = lanep.tile([P, P, E], F16, tag="rt4")
                rp_b = bass.AP(
                    tensor=rp.tensor,
                    offset=rp.offset + tt * E,
                    ap=[list(rp.ap[0]), [0, P], [1, E]],
                )
                nc.vector.tensor_tensor(
                    out=rt4, in0=rp_b, in1=iota4_f16, op=ALU.is_equal
                )
                for e in range(E):
                    rt_e = bass.AP(
                        tensor=rt4.tensor,
                        offset=rt4.offset + e,
                        ap=[list(rt4.ap[0]), [E, P]],
                    )
                    nc.tensor.matmul(
                        listps[e][:, 0:TAU],
                        rt_e,
                        idm[:, tt, e, :],
                        start=(tt == 0),
                        stop=(tt == TT - 1),
                    )
            for e in range(E):
                nc.vector.tensor_copy(out=lists_i[:, e, :], in_=listps[e][:, 0:TAU])
            lanep_ctx.__exit__(None, None, None)
            l_ctx.__exit__(None, None, None)

            # ------- dispatch gathers (f32 rows from x, single-chunk) -------
            xg_tiles = {}
            for e in range(E):
                for tau in range(TAU):
                    xg = xgp.tile([P, D], F32, tag="xg", name=f"xg_{e}_{tau}")
                    nc.gpsimd.indirect_dma_start(
                        out=xg,
                        out_offset=None,
                        in_=x_d[:, :],
                        in_offset=bass.IndirectOffsetOnAxis(
                            ap=lists_i[:, e, tau : tau + 1], axis=0
                        ),
                    )
                    xg_tiles[(e, tau)] = xg

            # ---------------- expert FFNs (software-pipelined) --------------
            e_ctx = tc.tile_pool(name="epsum", bufs=1, space="PSUM")
            ep = e_ctx.__enter__()
            zp_ctx = tc.tile_pool(name="zp", bufs=2, space="PSUM")
            zp = zp_ctx.__enter__()
            z2p_ctx = tc.tile_pool(name="z2p", bufs=1, space="PSUM")
            z2p = z2p_ctx.__enter__()

            NT = E * TAU
            us = {}
            xgTs = {}
            uTs = {}
            xgcs = {}

            def stage_cast(i):
                e, tau = divmod(i, TAU)
                xgc = workp.tile([P, D], BF16, tag="xgc")
                nc.scalar.copy(out=xgc, in_=xg_tiles[(e, tau)])
                xgcs[i] = xgc

            def stage_xgt(i):
                xgc = xgcs.pop(i)
                xtps = ep.tile([P, D], BF16, tag="xgt")
                for c in range(KC):
                    nc.tensor.transpose(xtps[:, ts(c, P)], xgc[:, ts(c, P)], id_bf16)
                xgT = workp.tile([P, KC, P], BF16, tag="xgT")
                xtv = xtps.rearrange("p (c q) -> p c q", c=KC)
                nc.scalar.copy(out=xgT[:, 0 : KC // 2, :], in_=xtv[:, 0 : KC // 2, :])
                nc.vector.tensor_copy(
                    out=xgT[:, KC // 2 :, :], in_=xtv[:, KC // 2 :, :]
                )
                xgTs[i] = xgT

            def stage_ut(i):
                u = us.pop(i)
                utps = ep.tile([P, D], BF16, tag="ut")
                for c in range(KC):
                    nc.tensor.transpose(utps[:, ts(c, P)], u[:, ts(c, P)], id_bf16)
                uT = workp.tile([P, KC, P], BF16, tag="uT")
                utv = utps.rearrange("p (c q) -> p c q", c=KC)
                nc.scalar.copy(out=uT[:, 0 : KC // 2, :], in_=utv[:, 0 : KC // 2, :])
                nc.vector.tensor_copy(
                    out=uT[:, KC // 2 :, :], in_=utv[:, KC // 2 :, :]
                )
                uTs[i] = uT

            def stage_z(i):
                e, tau = divmod(i, TAU)
                if tau == 0 and e + 1 < E:
                    load_w1(e + 1)
                if tau == 2 and e + 1 < E:
                    load_w2(e + 1)
                xgT = xgTs.pop(i)
                z = zp.tile([P, D], F32, tag="z")
                w1t = w1tiles[e]
                for c in range(KC):
                    for n in range(2):
                        nc.tensor.matmul(
                            z[:, ds(n * 512, 512)],
                            xgT[:, c, :],
                            w1t[:, c, ds(n * 512, 512)],
                            start=(c == 0),
                            stop=(c == KC - 1),
                        )
                st1 = statp.tile([P, 2, 6], F32, tag="st1")
                nc.vector.bn_stats(out=st1[:, 0, :], in_=z[:, 0:512])
                nc.vector.bn_stats(out=st1[:, 1, :], in_=z[:, 512:1024])
                mv1 = statp.tile([P, 2], F32, tag="mv1")
                nc.vector.bn_aggr(out=mv1, in_=st1)
                sd1 = statp.tile([P, 1], F32, tag="sd1")
                nc.scalar.activation(out=sd1, in_=mv1[:, 1:2], func=AF.Sqrt, bias=eps_sb)
                rs1 = statp.tile([P, 1], F32, tag="rs1")
                nc.vector.reciprocal(out=rs1, in_=sd1)
                nmr1 = statp.tile([P, 1], F32, tag="nmr1")
                nc.vector.tensor_scalar(
                    out=nmr1,
                    in0=mv1[:, 0:1],
                    scalar1=rs1,
                    scalar2=-1.0,
                    op0=ALU.mult,
                    op1=ALU.mult,
                )
                # u = relu((z - m) * rstd)  [g1=1, be1=0]
                u = workp.tile([P, D], BF16, tag="u")
                nc.scalar.activation(out=u, in_=z, func=AF.Relu, bias=nmr1, scale=rs1)
                us[i] = u

            def stage_z2(i):
                e, tau = divmod(i, TAU)
                uT = uTs.pop(i)
                z2 = z2p.tile([P, D], F32, tag="z2")
                w2t = w2tiles[e]
                for c in range(KC):
                    for n in range(2):
                        nc.tensor.matmul(
                            z2[:, ds(n * 512, 512)],
                            uT[:, c, :],
                            w2t[:, c, ds(n * 512, 512)],
                            start=(c == 0),
                            stop=(c == KC - 1),
                        )
                st2 = statp.tile([P, 2, 6], F32, tag="st2")
                nc.vector.bn_stats(out=st2[:, 0, :], in_=z2[:, 0:512])
                nc.vector.bn_stats(out=st2[:, 1, :], in_=z2[:, 512:1024])
                mv2 = statp.tile([P, 2], F32, tag="mv2")
                nc.vector.bn_aggr(out=mv2, in_=st2)
                sd2 = statp.tile([P, 1], F32, tag="sd2")
                nc.scalar.activation(out=sd2, in_=mv2[:, 1:2], func=AF.Sqrt, bias=eps_sb)
                rs2 = statp.tile([P, 1], F32, tag="rs2")
                nc.vector.reciprocal(out=rs2, in_=sd2)
                nmr2 = statp.tile([P, 1], F32, tag="nmr2")
                nc.vector.tensor_scalar(
                    out=nmr2,
                    in0=mv2[:, 0:1],
                    scalar1=rs2,
                    scalar2=-1.0,
                    op0=ALU.mult,
                    op1=ALU.mult,
                )
                # y = (z2 - m2) * rstd2  [g2=1, be2=0; weight applied at combine]
                y = workp.tile([P, D], BF16, tag="y")
                nc.scalar.activation(
                    out=y, in_=z2, func=AF.Identity, bias=nmr2, scale=rs2
                )
                nc.sync.dma_start(out=y_d[ds((e * TAU + tau) * P, P), :], in_=y)

            # PE order per iteration: ut(i-1), xgt(i+1), z(i), z2(i-1);
            # ACT casts run 2 iterations ahead.
            stage_cast(0)
            stage_cast(1)
            stage_xgt(0)
            for i in range(NT + 1):
                if i + 2 < NT:
                    stage_cast(i + 2)
                if 0 <= i - 1 < NT and (i - 1) in us:
                    stage_ut(i - 1)
                if i + 1 < NT:
                    stage_xgt(i + 1)
                if i < NT:
                    stage_z(i)
                if i >= 1:
                    stage_z2(i - 1)
            z2p_ctx.__exit__(None, None, None)
            zp_ctx.__exit__(None, None, None)
            e_ctx.__exit__(None, None, None)

            # ---------------- combine ----------------
            combp_ctx = tc.tile_pool(name="combp", bufs=3)
            combp = combp_ctx.__enter__()
            combp2_ctx = tc.tile_pool(name="combp2", bufs=2)
            combp2 = combp2_ctx.__enter__()
            for tt in range(TT):
                yA = combp.tile([P, D], BF16, tag="yA")
                nc.gpsimd.indirect_dma_start(
                    out=yA,
                    out_offset=None,
                    in_=y_d[:, :],
                    in_offset=bass.IndirectOffsetOnAxis(
                        ap=sAB_i[:, tt, 0:1], axis=0
                    ),
                )
                yB = combp.tile([P, D], BF16, tag="yB")
                nc.gpsimd.indirect_dma_start(
                    out=yB,
                    out_offset=None,
                    in_=y_d[:, :],
                    in_offset=bass.IndirectOffsetOnAxis(
                        ap=sAB_i[:, tt, 1:2], axis=0
                    ),
                )
                a = combp2.tile([P, D], BF16, tag="a")
                nc.scalar.activation(
                    out=a, in_=yA, func=AF.Identity, scale=wAB[:, tt, 0:1]
                )
                b = combp2.tile([P, D], BF16, tag="b")
                nc.scalar.activation(
                    out=b, in_=yB, func=AF.Identity, scale=wAB[:, tt, 1:2]
                )
                nc.vector.tensor_tensor(out=a, in0=a, in1=b, op=ALU.add)
                t2 = workp.tile([P, D], F32, tag="t2")
                nc.vector.tensor_tensor(
                    out=t2, in0=a, in1=x_res[:, tt, :], op=ALU.add
                )
                nc.sync.dma_start(out=out_d[ts(tt, P), :], in_=t2)
            combp2_ctx.__exit__(None, None, None)
            combp_ctx.__exit__(None, None, None)

    nc.compile()
    return nc


_nc_cache = {}
_nc_lock = threading.Lock()


def _get_nc(T, num_devices, kind="routed"):
    key = (T, num_devices, kind)
    with _nc_lock:
        if key not in _nc_cache:
            if kind == "routed":
                _nc_cache[key] = build_moe_routed_nc(T, num_devices)
            else:
                _nc_cache[key] = build_moe_nc(T, num_devices)
        return _nc_cache[key]


def _fast_path_ok(inputs, x):
    """Fast path requires the zero/one parameter specializations and that
    no (core, expert) top-2 assignment count exceeds CAP."""
    try:
        if not (
            np.all(inputs["gate_b"] == 0.0)
            and np.all(inputs["b1"] == 0.0)
            and np.all(inputs["b2"] == 0.0)
            and np.all(inputs["be1"] == 0.0)
            and np.all(inputs["be2"] == 0.0)
            and np.all(inputs["g1"] == 1.0)
            and np.all(inputs["g2"] == 1.0)
        ):
            return False
        gw = np.asarray(inputs["gate_W"], dtype=np.float32)
        scores = x.reshape(-1, x.shape[-1]) @ gw  # [B*N, E]
        B, N, _ = x.shape
        E = gw.shape[1]
        order = np.argsort(-scores, axis=-1)[:, :2]
        for b in range(B):
            sel = order[b * N : (b + 1) * N]
            counts = np.bincount(sel.reshape(-1), minlength=E)
            # margin of 8 guards against host/device fp32 tie-break skew
            if counts.max() > CAP - 8:
                return False
        return True
    except Exception:
        return False


def kernel(**inputs) -> np.ndarray:
    from concourse.bass_utils import run_bass_kernel_spmd

    x = np.ascontiguousarray(np.asarray(inputs["x"], dtype=np.float32))
    B, N, Dd = x.shape
    assert Dd == D and B == N_CORES, (B, N, Dd)

    if _fast_path_ok(inputs, x):
        nc = _get_nc(N, N_CORES, "routed")
        weights = {
            k: np.ascontiguousarray(np.asarray(inputs[k], dtype=np.float32))
            for k in ("gate_W", "W1", "W2")
        }
        in_maps = [dict(weights, x=x[i]) for i in range(N_CORES)]
    else:
        nc = _get_nc(N, N_CORES, "dense")
        weights = {
            k: np.ascontiguousarray(np.asarray(inputs[k], dtype=np.float32))
            for k in (
                "gate_W", "gate_b", "W1", "b1", "g1", "be1",
                "W2", "b2", "g2", "be2",
            )
        }
        in_maps = [dict(weights, x=x[i]) for i in range(N_CORES)]
    res = run_bass_kernel_spmd(nc, in_maps, core_ids=list(range(N_CORES)))
    out = np.stack([r["out"] for r in res.results], axis=0)
    return out.astype(np.float32)
